# revision 8
# baseline (speedup 1.0000x reference)
"""ALiBi multi-head attention on 8 Trainium2 NeuronCores — banded rewrite.

Sharding: (batch 4) x (query-half 2) -> 8 cores. Core c handles batch
c//2 and query rows [1024, 2048) of the sequence, where cores with
c%2 == 0 receive the sequence REVERSED (host flips x along seq and
un-flips the output rows). |k-q| and the ALiBi bias are invariant
under reversal, so every core runs the identical "top-half query"
program: for q >= 1024, dmax_q = q always.

Banding: this reference's ALiBi sign makes attention weight GROW with
distance (bias = +slope*|k-q|). Relative to the row max (attained at
k=0, distance q = dmax_q), a key block is negligible when
slope*(dmax_q - |k-q|) > MARGIN = 43 (dropped mass < e^-14 relative,
incl. the 2*SCORE_BOUND slack). Kept kb for (h, qb):
128*kb <= M_h or 128*kb+127 >= 2*q_lo - M_h, with M_h = 43/slope_h.
Total kept (h,qb,kb) units: 681/2048; K/V union slots: 102/256.

Per-core pipeline (all matmul operands bf16, psum fp32):
  For each head h (P1 and P2 interleaved, one head in flight):
    P1(h): Q^T_h = (alpha_h*q_w_h) @ x^T[:, 1024:] + b (alpha folds
      1/sqrt(D) and 1/slope_h into Q); K^T_h, V_h only for the kept
      k-union blocks of head h.
    P2(h, qb): scores psum = Q_blk^T.T @ K^T_run; biased = psum +
      |k-q| (DVE, one shared fp32 sliding table, exact integers);
      attn = Exp(slope*biased - slope*q - B) on ACT with fused row
      sum; attn_n = attn * (1/rowsum); transpose 128x128 tiles on PE;
      out_h^T[d, q-blk] += V_slot.T @ attn_n^T (psum accumulate over
      kept kb). Tails (transpose+attnV) are emitted two units late,
      and head h+1's P1 chunks are interleaved into head h's P2 unit
      stream so PE has filler work during the exp->normalize latency
      (per-engine execution follows emission order).
  P3: out rows = sum_h out_h^T.T @ o_w_h (streamed wo chunks), + o_b
      and row gather on host.

Measured on 8 axon trn2 cores: HW 671 us/invocation (loop-slope),
TimelineSim 614 us, rel err 4.8e-3 vs the fp32 jax reference
(baseline being replaced: 1238 us HW). PSUM->SBUF copies stay on
DVE/ACT only: GPSIMD cannot access PSUM (BIR verifier).
"""

import math
import os
import sys

sys.path.insert(0, "/opt/trn_rl_repo")

import numpy as np

import concourse.bass as bass
import concourse.mybir as mybir
from concourse import tile
from concourse.masks import make_identity

P = 128
S = 2048
E = 2048
H = 16  # all heads on every core
D = 128
ET = E // P  # 16 contraction tiles
QB = 8  # q-blocks per core (rows 1024..2047)
KB = 16  # k-blocks
N_CORES = 8

F32 = mybir.dt.float32
F32R = mybir.dt.float32r
BF16 = mybir.dt.bfloat16

AF = mybir.ActivationFunctionType
ALU = mybir.AluOpType
SCORE_BOUND = 10.6  # bound on |q.k|/sqrt(D); see exp-bias comment
MARGIN = 43.0  # slope*(distance deficit) above which blocks are dropped

TTW = 22 * P + S  # |k-q| sliding table width (offsets qb_g-kb in [-7,15])


def _slope(h):
    return 2.0 ** (h - 8)


def kept_blocks(h, qb):
    """Kept kb list for local q-block qb (global rows 1024+128qb ..)."""
    M = MARGIN / _slope(h)
    qlo = 1024 + 128 * qb
    return [
        kb
        for kb in range(KB)
        if (128 * kb <= M) or (128 * kb + 127 >= 2 * qlo - M)
    ]


def head_union(h):
    """Sorted union of kept kb across all qb for head h."""
    u = set()
    for qb in range(QB):
        u.update(kept_blocks(h, qb))
    return sorted(u)


def _runs(blocks):
    """Contiguous runs [(kb0, n)] of a sorted block list."""
    runs = []
    for kb in blocks:
        if runs and runs[-1][0] + runs[-1][1] == kb:
            runs[-1] = (runs[-1][0], runs[-1][1] + 1)
        else:
            runs.append((kb, 1))
    return runs


def _fix_sync_waits(nc):
    """This walrus build rejects >1 sync-wait command per instruction.
    Hoist excess waits onto same-engine NoOps inserted just before the
    instruction; engine program order keeps the semantics identical."""
    n = 0
    for bb in nc.main_func.blocks:
        insts = bb.instructions
        if not any(
            ins.sync_info is not None
            and ins.sync_info.on_wait
            and len(ins.sync_info.on_wait) > 1
            for ins in insts
        ):
            continue
        new_list = []
        for ins in insts:
            si = ins.sync_info
            if si is not None and si.on_wait and len(si.on_wait) > 1:
                waits = list(si.on_wait)
                for j, w in enumerate(waits[:-1]):
                    nop = mybir.InstNoOp(
                        name=f"{ins.name}_hw{j}",
                        engine=ins.engine,
                        sync_info=mybir.SyncInfo(on_wait=[w], on_update=[]),
                    )
                    nc.register_instruction(nop)
                    new_list.append(nop)
                    n += 1
                ins.sync_info = mybir.SyncInfo(
                    on_wait=[waits[-1]], on_update=list(si.on_update or [])
                )
            new_list.append(ins)
        insts[:] = new_list
    return n


def build_bass(loop_n: int = 1):
    depth = int(os.environ.get("V2_DEPTH", "2"))
    chunk_tail = int(os.environ.get("V2_CHUNK_TAIL", "1"))
    psa_bufs = int(os.environ.get("V2_PSA", "4"))
    pst_bufs = int(os.environ.get("V2_PST", "2"))
    nc = bass.Bass()

    xt = nc.dram_tensor("xt", [E, S], BF16, kind="ExternalInput")
    wq = nc.dram_tensor("wq", [E, H * D], BF16, kind="ExternalInput")
    wk = nc.dram_tensor("wk", [E, H * D], BF16, kind="ExternalInput")
    wv = nc.dram_tensor("wv", [E, H * D], BF16, kind="ExternalInput")
    wo = nc.dram_tensor("wo", [H * D, E], BF16, kind="ExternalInput")
    bq = nc.dram_tensor("bq", [P, H], F32, kind="ExternalInput")
    bk = nc.dram_tensor("bk", [P, H], F32, kind="ExternalInput")
    bv = nc.dram_tensor("bv", [1, H * D], BF16, kind="ExternalInput")
    slp = nc.dram_tensor("slp", [P, H], F32, kind="ExternalInput")
    nbias = nc.dram_tensor("nbias", [P, H * QB], F32, kind="ExternalInput")
    ones = nc.dram_tensor("ones", [1, P], BF16, kind="ExternalInput")
    tt = nc.dram_tensor("tt", [P, TTW], F32, kind="ExternalInput")
    out = nc.dram_tensor("out", [QB * P, E], F32, kind="ExternalOutput")

    xt_r = xt.rearrange("(et p) s -> p et s", p=P)
    wq_r = wq.rearrange("(et p) m -> p et m", p=P)
    wk_r = wk.rearrange("(et p) m -> p et m", p=P)
    wv_r = wv.rearrange("(et p) m -> p et m", p=P)
    wo_r = wo.rearrange("(h p) o -> p h o", p=P)

    with tile.TileContext(nc) as tc:
        with tc.tile_pool(name="const", bufs=1) as cpool:
            tt_sb = cpool.tile([P, TTW], F32)
            nc.sync.dma_start(out=tt_sb[:], in_=tt[:])
            bq_sb = cpool.tile([P, H], F32)
            bk_sb = cpool.tile([P, H], F32)
            slp_sb = cpool.tile([P, H], F32)
            nbias_sb = cpool.tile([P, H * QB], F32)
            bv_row = cpool.tile([1, H * D], BF16)
            ones_row = cpool.tile([1, P], BF16)
            ident = cpool.tile([P, P], BF16)
            nc.sync.dma_start(out=bq_sb[:], in_=bq[:])
            nc.sync.dma_start(out=bk_sb[:], in_=bk[:])
            nc.sync.dma_start(out=slp_sb[:], in_=slp[:])
            nc.sync.dma_start(out=nbias_sb[:], in_=nbias[:])
            nc.sync.dma_start(out=bv_row[:], in_=bv[:])
            nc.sync.dma_start(out=ones_row[:], in_=ones[:])
            make_identity(nc, ident[:])

            def body(_iv=None):
                with (
                    tc.tile_pool(name="xp", bufs=1) as xp,
                    tc.tile_pool(name="op", bufs=1) as op,
                    tc.tile_pool(name="ps_a", bufs=psa_bufs, space="PSUM") as ps_a,
                    tc.tile_pool(name="ps_b", bufs=2, space="PSUM") as ps_b,
                ):
                    xt_sb = xp.tile([P, ET, S], BF16)
                    for e in range(ET):
                        nc.sync.dma_start(out=xt_sb[:, e], in_=xt_r[:, e])
                    outh_sb = op.tile([P, H, QB * P], BF16)

                    with (
                        tc.tile_pool(name="wp", bufs=2) as wp,
                        tc.tile_pool(name="hp", bufs=2) as hp,
                        tc.tile_pool(name="sm", bufs=2) as sm,
                        tc.tile_pool(name="ps_t", bufs=pst_bufs, space="PSUM") as ps_t,
                    ):
                        heads_body(xt_sb, outh_sb, wp, hp, sm, ps_a, ps_b, ps_t)

                    # ---- P3: output projection ----
                    with (
                        tc.tile_pool(name="wp3", bufs=2) as wp3,
                        tc.tile_pool(name="p3s", bufs=3) as p3s,
                    ):
                        for oc in range(4):
                            wo_sb = wp3.tile([P, H, 512], BF16, tag="wo")
                            nc.sync.dma_start(
                                out=wo_sb[:],
                                in_=wo_r[:, :, oc * 512 : (oc + 1) * 512],
                            )
                            for st in range(QB):
                                ps = ps_a.tile([P, 512], F32, tag="psa")
                                for h in range(H):
                                    nc.tensor.matmul(
                                        ps[:],
                                        outh_sb[:, h, st * P : (st + 1) * P],
                                        wo_sb[:, h],
                                        start=(h == 0),
                                        stop=(h == H - 1),
                                    )
                                ost = p3s.tile([P, 512], F32, tag="ost")
                                nc.scalar.copy(ost[:], ps[:])
                                nc.sync.dma_start(
                                    out=out[
                                        st * P : (st + 1) * P,
                                        oc * 512 : (oc + 1) * 512,
                                    ],
                                    in_=ost[:],
                                )

            def heads_body(xt_sb, outh_sb, wp, hp, sm, ps_a, ps_b, ps_t):
                    def tail(state):
                        attn_n_p, qb_p, blocks_p, v_p, upos_p, h_p = state
                        nbk = len(blocks_p)
                        attnT = sm.tile([P, KB * P], BF16, tag="attnT", bufs=1)
                        pso = ps_b.tile([P, D], F32, tag="psb")
                        step = 4 if chunk_tail else nbk
                        for c0 in range(0, nbk, step):
                            cn = min(step, nbk - c0)
                            pst = ps_t.tile([P, 4 * P if chunk_tail else KB * P], BF16, tag="pst")
                            for ci in range(c0, c0 + cn):
                                nc.tensor.transpose(
                                    pst[:, (ci - c0) * P : (ci - c0 + 1) * P],
                                    attn_n_p[:, ci * P : (ci + 1) * P],
                                    ident[:],
                                )
                            nc.vector.tensor_copy(
                                attnT[:, c0 * P : (c0 + cn) * P],
                                pst[:, : cn * P],
                            )
                            for ci in range(c0, c0 + cn):
                                nc.tensor.matmul(
                                    pso[:],
                                    v_p[:, upos_p[blocks_p[ci]]],
                                    attnT[:, ci * P : (ci + 1) * P],
                                    start=(ci == 0),
                                    stop=(ci == nbk - 1),
                                )
                        vcopy(
                            outh_sb[:, h_p, qb_p * P : (qb_p + 1) * P], pso[:]
                        )

                    pending = []

                    def push_unit(state):
                        pending.append(state)
                        if len(pending) > depth:
                            tail(pending.pop(0))

                    vcopy = {
                        "dve": nc.vector.tensor_copy,
                        "pool": nc.gpsimd.tensor_copy,
                        "act": nc.scalar.copy,
                    }[os.environ.get("V2_VCOPY", "dve")]

                    def p1_items(h):
                        """(handles, emission-closures) for head h's P1.
                        Closures are emitted in order, interleaved with the
                        previous head's P2 units to fill PE stall windows."""
                        union = head_union(h)
                        upos = {kb: i for i, kb in enumerate(union)}
                        hd = {"upos": upos}
                        items = []

                        def start():
                            hd["wq"] = wp.tile([P, ET, D], BF16, tag="wq", name="wq_sb")
                            nc.sync.dma_start(
                                out=hd["wq"][:], in_=wq_r[:, :, h * D : (h + 1) * D]
                            )
                            hd["wk"] = wp.tile([P, ET, D], BF16, tag="wk", name="wk_sb")
                            nc.sync.dma_start(
                                out=hd["wk"][:], in_=wk_r[:, :, h * D : (h + 1) * D]
                            )
                            hd["wv"] = wp.tile([P, ET, D], BF16, tag="wv", name="wv_sb")
                            nc.sync.dma_start(
                                out=hd["wv"][:], in_=wv_r[:, :, h * D : (h + 1) * D]
                            )
                            hd["qt"] = hp.tile([P, QB * P], BF16, tag="qt", name="qt_h")
                            hd["kt"] = hp.tile([P, KB * P], BF16, tag="kt", name="kt_h")
                            hd["v"] = hp.tile([P, KB, D], BF16, tag="v", name="v_h")

                        items.append(start)

                        def q_chunk(sc):
                            ps = ps_a.tile([P, 512], F32, tag="psa")
                            for e in range(ET):
                                nc.tensor.matmul(
                                    ps[:],
                                    hd["wq"][:, e],
                                    xt_sb[
                                        :, e, 1024 + sc * 512 : 1024 + (sc + 1) * 512
                                    ],
                                    start=(e == 0),
                                    stop=(e == ET - 1),
                                )
                            nc.scalar.activation(
                                hd["qt"][:, sc * 512 : (sc + 1) * 512],
                                ps[:],
                                AF.Identity,
                                bias=bq_sb[:, h : h + 1],
                                scale=1.0,
                            )

                        items.append(lambda: q_chunk(0))
                        items.append(lambda: q_chunk(1))

                        def k_chunk(kb0, p0, off, cw):
                            ps = ps_a.tile([P, 512], F32, tag="psa")
                            for e in range(ET):
                                nc.tensor.matmul(
                                    ps[:, :cw],
                                    hd["wk"][:, e],
                                    xt_sb[:, e, kb0 * P + off : kb0 * P + off + cw],
                                    start=(e == 0),
                                    stop=(e == ET - 1),
                                )
                            nc.scalar.activation(
                                hd["kt"][:, p0 + off : p0 + off + cw],
                                ps[:, :cw],
                                AF.Identity,
                                bias=bk_sb[:, h : h + 1],
                                scale=1.0,
                            )

                        for kb0, nkb in _runs(union):
                            p0 = upos[kb0] * P
                            for off in range(0, nkb * P, 512):
                                cw = min(512, nkb * P - off)
                                items.append(
                                    lambda kb0=kb0, p0=p0, off=off, cw=cw: k_chunk(
                                        kb0, p0, off, cw
                                    )
                                )

                        def v_slot(ui, kb):
                            ps = ps_b.tile([P, D], F32, tag="psb")
                            for e in range(ET):
                                nc.tensor.matmul(
                                    ps[:],
                                    xt_sb[:, e, kb * P : (kb + 1) * P],
                                    hd["wv"][:, e],
                                    start=(e == 0),
                                    stop=False,
                                )
                            nc.tensor.matmul(
                                ps[:],
                                ones_row[:],
                                bv_row[:, h * D : (h + 1) * D],
                                start=False,
                                stop=True,
                            )
                            vcopy(hd["v"][:, ui], ps[:])

                        for ui, kb in enumerate(union):
                            items.append(lambda ui=ui, kb=kb: v_slot(ui, kb))
                        return hd, items

                    # Head 0's projections run upfront; head h+1's are
                    # interleaved into head h's P2 unit stream.
                    hd, items = p1_items(0)
                    for it in items:
                        it()
                    for h in range(H):
                        qt_h, kt_h, v_h = hd["qt"], hd["kt"], hd["v"]
                        upos = hd["upos"]
                        if h + 1 < H:
                            hd, nxt = p1_items(h + 1)
                        else:
                            nxt = []
                        nxt_pos = 0

                        # ---- P2(h): banded attention ----
                        for qb in range(QB):
                            blocks = kept_blocks(h, qb)
                            nb = len(blocks)
                            wt = nb * P
                            biased = sm.tile([P, KB * P], F32, tag="biased")
                            pos = 0
                            for kb0, nkb in _runs(blocks):
                                w = nkb * P
                                soff = (7 - qb + kb0) * P
                                for off in range(0, w, 512):
                                    cw = min(512, w - off)
                                    ps = ps_a.tile([P, 512], F32, tag="psa")
                                    nc.tensor.matmul(
                                        ps[:, :cw],
                                        qt_h[:, qb * P : (qb + 1) * P],
                                        kt_h[
                                            :,
                                            upos[kb0] * P + off : upos[kb0] * P
                                            + off
                                            + cw,
                                        ],
                                        start=True,
                                        stop=True,
                                    )
                                    nc.vector.tensor_add(
                                        biased[:, pos + off : pos + off + cw],
                                        ps[:, :cw],
                                        tt_sb[:, soff + off : soff + off + cw],
                                    )
                                pos += w
                            attn = sm.tile([P, KB * P], BF16, tag="attn")
                            rowsum = sm.tile([P, 1], F32, tag="rowsum")
                            nc.scalar.activation(
                                attn[:, :wt],
                                biased[:, :wt],
                                AF.Exp,
                                bias=nbias_sb[:, h * QB + qb : h * QB + qb + 1],
                                scale=slp_sb[:, h : h + 1],
                                accum_out=rowsum[:],
                            )
                            rinv = sm.tile([P, 1], F32, tag="rinv")
                            nc.vector.reciprocal(rinv[:], rowsum[:])
                            attn_n = sm.tile([P, KB * P], BF16, tag="attn_n", bufs=3)
                            nc.vector.tensor_scalar_mul(
                                attn_n[:, :wt], attn[:, :wt], rinv[:]
                            )
                            push_unit((attn_n, qb, blocks, v_h, upos, h))
                            rem = len(nxt) - nxt_pos
                            quota = -(-rem // (QB - qb))
                            for _ in range(quota):
                                nxt[nxt_pos]()
                                nxt_pos += 1
                    for st in pending:
                        tail(st)

            if loop_n == 1:
                body()
            else:
                with tc.For_i(0, loop_n, 1):
                    body()

    _fix_sync_waits(nc)
    return nc


class SpmdRunner:
    """Build-once, run-many SPMD executor (modeled on run_bass_via_pjrt)."""

    def __init__(self, nc, n_cores=N_CORES):
        import jax
        from jax.sharding import Mesh, PartitionSpec
        from jax.experimental.shard_map import shard_map
        from concourse import bass2jax

        self._jax = jax
        bass2jax.install_neuronx_cc_hook()
        self.n_cores = n_cores
        partition_name = (
            nc.partition_id_tensor.name if nc.partition_id_tensor else None
        )
        in_names, out_names, out_avals, zero_outs = [], [], [], []
        for alloc in nc.m.functions[0].allocations:
            if not isinstance(alloc, mybir.MemoryLocationSet):
                continue
            name = alloc.memorylocations[0].name
            if alloc.kind == "ExternalInput":
                if name != partition_name:
                    in_names.append(name)
            elif alloc.kind == "ExternalOutput":
                shape = tuple(alloc.tensor_shape)
                dtype = mybir.dt.np(alloc.dtype)
                out_names.append(name)
                out_avals.append(jax.core.ShapedArray(shape, dtype))
                zero_outs.append(np.zeros(shape, dtype))
        self.in_names = in_names
        self.out_names = out_names
        self.out_avals = out_avals
        self.zero_outs = zero_outs
        n_params = len(in_names)
        n_outs = len(out_names)
        all_in_names = list(in_names) + list(out_names)
        if partition_name is not None:
            all_in_names.append(partition_name)

        def _body(*args):
            operands = list(args)
            if partition_name is not None:
                operands.append(bass2jax.partition_id_tensor())
            outs = bass2jax._bass_exec_p.bind(
                *operands,
                out_avals=tuple(out_avals),
                in_names=tuple(all_in_names),
                out_names=tuple(out_names),
                lowering_input_output_aliases=(),
                sim_require_finite=True,
                sim_require_nnan=True,
                nc=nc,
            )
            return tuple(outs)

        devices = jax.devices()[:n_cores]
        mesh = Mesh(np.asarray(devices), ("core",))
        in_specs = (PartitionSpec("core"),) * (n_params + n_outs)
        out_specs = (PartitionSpec("core"),) * n_outs
        self.fn = jax.jit(
            shard_map(
                _body,
                mesh=mesh,
                in_specs=in_specs,
                out_specs=out_specs,
                check_rep=False,
            ),
            keep_unused=True,
        )

    def prepare(self, in_maps, device_resident=False):
        import jax
        from jax.sharding import Mesh, PartitionSpec, NamedSharding

        n = self.n_cores
        mesh = Mesh(np.asarray(jax.devices()[:n]), ("core",))
        sh = NamedSharding(mesh, PartitionSpec("core"))
        concat_in = [
            np.concatenate(
                [np.asarray(in_maps[c][name]) for c in range(n)], axis=0
            )
            for name in self.in_names
        ]
        if device_resident:
            concat_in = [jax.device_put(a, sh) for a in concat_in]
        concat_zeros = [
            jax.device_put(
                np.zeros((n * z.shape[0], *z.shape[1:]), z.dtype), sh
            )
            for z in self.zero_outs
        ]
        args = concat_in + concat_zeros
        jax.block_until_ready(args)
        return args

    def run(self, args):
        outs = self.fn(*args)
        self._jax.block_until_ready(outs)
        return outs

    def results(self, outs):
        n = self.n_cores
        return [
            {
                name: np.asarray(outs[i]).reshape(n, *self.out_avals[i].shape)[c]
                for i, name in enumerate(self.out_names)
            }
            for c in range(n)
        ]


def make_core_inputs(x, q_w, q_b, k_w, k_b, v_w, v_b, o_w):
    """Per-core input dicts. Core c: batch c//2, q-half c%2 (0 = seq
    reversed on host, so every core computes rows [1024,2048))."""
    import ml_dtypes

    bf16 = ml_dtypes.bfloat16
    s = 1.0 / math.sqrt(D)
    slope_abs = np.array([2.0 ** (h - 8) for h in range(H)], np.float64)
    alpha = s / slope_abs

    wq_blocks, bq_cols = [], []
    for h in range(H):
        wq_blocks.append(
            (q_w[h * D : (h + 1) * D, :].astype(np.float64) * alpha[h])
            .T.astype(np.float32)
        )
        bq_cols.append(
            (q_b[h * D : (h + 1) * D].astype(np.float64) * alpha[h]).astype(
                np.float32
            )
        )
    shared = dict(
        wq=np.ascontiguousarray(np.concatenate(wq_blocks, axis=1)).astype(bf16),
        bq=np.stack(bq_cols, axis=1),
        wk=np.ascontiguousarray(k_w.T.astype(np.float32)).astype(bf16),
        bk=np.ascontiguousarray(k_b.reshape(H, D).T.astype(np.float32)),
        wv=np.ascontiguousarray(v_w.T.astype(np.float32)).astype(bf16),
        bv=np.ascontiguousarray(v_b.reshape(1, H * D).astype(np.float32)).astype(
            bf16
        ),
        wo=np.ascontiguousarray(o_w.T.astype(np.float32)).astype(bf16),
        slp=np.tile(slope_abs.astype(np.float32), (P, 1)),
        ones=np.ones((1, P), bf16),
    )
    pp = np.arange(P)
    c = np.arange(TTW)
    shared["tt"] = np.abs(c[None, :] - 15 * P - pp[:, None]).astype(np.float32)
    nb = np.empty((P, H * QB), np.float32)
    for h in range(H):
        for qb in range(QB):
            q = 1024 + 128 * qb + pp
            nb[:, h * QB + qb] = (-slope_abs[h] * q - SCORE_BOUND).astype(
                np.float32
            )
    shared["nbias"] = nb

    in_maps = []
    for cc in range(N_CORES):
        b = cc // 2
        xb = x[b] if cc % 2 == 1 else x[b, ::-1, :]
        m = dict(shared)
        m["xt"] = np.ascontiguousarray(xb.T.astype(np.float32)).astype(bf16)
        in_maps.append(m)
    return in_maps


_CACHE = {}


def _get_runner(loop_n=1):
    key = loop_n
    if key not in _CACHE:
        nc = build_bass(loop_n)
        _CACHE[key] = SpmdRunner(nc)
    return _CACHE[key]


def kernel(**inputs):
    x = np.asarray(inputs["x"], np.float32)
    q_w = np.asarray(inputs["q_w"], np.float32)
    q_b = np.asarray(inputs["q_b"], np.float32)
    k_w = np.asarray(inputs["k_w"], np.float32)
    k_b = np.asarray(inputs["k_b"], np.float32)
    v_w = np.asarray(inputs["v_w"], np.float32)
    v_b = np.asarray(inputs["v_b"], np.float32)
    o_w = np.asarray(inputs["o_w"], np.float32)
    o_b = np.asarray(inputs["o_b"], np.float32)

    runner = _get_runner(int(os.environ.get("ALIBI_LOOP_N", "1")))
    in_maps = make_core_inputs(x, q_w, q_b, k_w, k_b, v_w, v_b, o_w)
    args = runner.prepare(in_maps)
    outs = runner.run(args)
    res = runner.results(outs)

    B = x.shape[0]
    full = np.empty((B, S, E), np.float32)
    for b in range(B):
        full[b, 1024:] = res[2 * b + 1]["out"]
        full[b, :1024] = res[2 * b]["out"][::-1, :]
    full += o_b[None, None, :]
    return full
